# revision 1
# baseline (speedup 1.0000x reference)
"""Trainium2 Bass kernel for the L1-distance attention + MLP-scaling model.

Math (per batch b):
  Wk = MLP(K), Wq = MLP(Q), Wo = MLPo(Q)
  Ks = K*Wk, Qs = Q*Wq
  score[k,q] = sum_d |Ks[k,d] - Qs[q,d]|
             = (Sq[q] - Sk[k]) + 2*sum_d relu(Ks[k,d] - Qs[q,d])
  attn = softmax_k(-(score^2)/2)
  out = (attn^T @ V) * Wo

Sharding: 8 cores = 4 batches x 2 query-halves. Each core handles all 4096
keys and 2048 queries of its batch.
"""
import sys
sys.path.insert(0, '/opt/trn_rl_repo')
import numpy as np
from contextlib import ExitStack

import concourse.bass as bass
import concourse.bacc as bacc
import concourse.tile as tile
from concourse import mybir
from concourse.bass_utils import run_bass_kernel_spmd

dt = mybir.dt
F32 = dt.float32
F32R = dt.float32r
ALU = mybir.AluOpType
AF = mybir.ActivationFunctionType
AX = mybir.AxisListType

B, NK, NQ, DK, DV, H = 4, 4096, 4096, 64, 64, 256
NCORES = 8
QSH = NQ // 2            # queries per core
NSUB = QSH // 128        # 16 q-subtiles of 128
KCH = NK // 128          # 32 key chunks
KB = NK // 512           # 8 psum banks of 512 keys
SQ2 = float(np.float32(1.0 / np.sqrt(2.0)))

_cache = {}


def _build(nsub, reps=1):
    nc = bacc.Bacc("TRN2", target_bir_lowering=False, debug=False,
                   num_devices=NCORES)

    def din(name, shape, d=F32):
        return nc.dram_tensor(name, shape, d, kind="ExternalInput").ap()

    kt = din("kt", [64, NK])            # Ks^T source: K^T
    qt = din("qt", [64, QSH])           # Q^T
    v1 = din("v1", [128, KCH * 65])     # [V | ones] packed per key-chunk
    w1 = din("w1", [64, H])             # W1_w^T
    w2a = din("w2a", [128, H]); w2b = din("w2b", [128, H])
    w3a = din("w3a", [128, DK]); w3b = din("w3b", [128, DK])
    b1c = din("b1c", [128, 2]); b2c = din("b2c", [128, 2]); b3c = din("b3c", [128, 1])
    u1 = din("u1", [64, H])
    u2a = din("u2a", [128, H]); u2b = din("u2b", [128, H])
    u3a = din("u3a", [128, DK]); u3b = din("u3b", [128, DK])
    c1c = din("c1c", [128, 2]); c2c = din("c2c", [128, 2]); c3c = din("c3c", [128, 1])
    em = din("em", [128, 256], F32R)    # shifted-master E (+2.0 blocks)
    iden = din("iden", [128, 128])
    O = nc.dram_tensor("o", [QSH, DV], F32, kind="ExternalOutput").ap()

    with tile.TileContext(nc) as tc:
        with ExitStack() as ctx:
            if reps > 1:
                ctx.enter_context(tc.For_i(0, reps, 1))
            sb = ctx.enter_context(tc.tile_pool(name="sb", bufs=1))
            hp = ctx.enter_context(tc.tile_pool(name="hp", bufs=2))
            bp = ctx.enter_context(tc.tile_pool(name="bp", bufs=1))
            rp = ctx.enter_context(tc.tile_pool(name="rp", bufs=2))
            pp = ctx.enter_context(tc.tile_pool(name="pp", bufs=1, space="PSUM"))

            def psum(tag):
                return pp.tile([128, 512], F32, tag=tag, name=tag)

            # ---------- load inputs ----------
            def load(ap_dram, shape, d=F32, tag=None):
                t = sb.tile(shape, d, tag=tag, name=tag)
                nc.gpsimd.dma_start(t[:], ap_dram)
                return t

            kt_t = load(kt, [64, NK], tag="kt")
            qt_t = load(qt, [64, QSH], tag="qt")
            v1_t = load(v1, [128, KCH * 65], tag="v1")
            iden_t = load(iden, [128, 128], tag="iden")
            em_t = load(em, [128, 256], F32R, tag="em_t")

            # weights: bounce via DVE so matmuls wait on <=1 DMA
            def wload(ap_dram, shape, tag):
                t = sb.tile(shape, F32, tag=tag, name=tag)
                nc.gpsimd.dma_start(t[:], ap_dram)
                return t

            w1_t = wload(w1, [64, H], "w1")
            w2a_t = wload(w2a, [128, H], "w2a"); w2b_t = wload(w2b, [128, H], "w2b")
            w3a_t = wload(w3a, [128, DK], "w3a"); w3b_t = wload(w3b, [128, DK], "w3b")
            u1_t = wload(u1, [64, H], "u1")
            u2a_t = wload(u2a, [128, H], "u2a"); u2b_t = wload(u2b, [128, H], "u2b")
            u3a_t = wload(u3a, [128, DK], "u3a"); u3b_t = wload(u3b, [128, DK], "u3b")
            b1_t = wload(b1c, [128, 2], "b1"); b2_t = wload(b2c, [128, 2], "b2")
            b3_t = wload(b3c, [128, 1], "b3")
            c1_t = wload(c1c, [128, 2], "c1"); c2_t = wload(c2c, [128, 2], "c2")
            c3_t = wload(c3c, [128, 1], "c3")

            # ---------- MLPs (transposed layout: features on partitions) ----------
            # out_t[0:64, :] gets scaled product written in-place for ks2
            ks2 = sb.tile([128, NK], F32, tag="ks2")
            qsct = sb.tile([64, QSH], F32, tag="qsct")
            wot = sb.tile([64, QSH], F32, tag="wot")

            def mlp(x_t, T, l1, l2a, l2b, l3a, l3b, bb1, bb2, bb3, out_ap,
                    scale_by=None):
                # x_t: [64, T] input^T; writes MLP output^T (64 rows) to out_ap
                # if scale_by is given, writes (mlp_out * scale_by) instead
                for c in range(T // 512):
                    xc = x_t[:, c * 512:(c + 1) * 512]
                    pa, pb = psum("bank0"), psum("bank1")
                    nc.tensor.matmul(pa[:], l1[:, 0:128], xc, start=True, stop=True)
                    nc.tensor.matmul(pb[:], l1[:, 128:256], xc, start=True, stop=True)
                    h1a = hp.tile([128, 512], F32, tag="h1a")
                    h1b = hp.tile([128, 512], F32, tag="h1b")
                    nc.vector.tensor_scalar(h1a[:], pa[:], bb1[:, 0:1], 0.0,
                                            ALU.add, ALU.max)
                    nc.vector.tensor_scalar(h1b[:], pb[:], bb1[:, 1:2], 0.0,
                                            ALU.add, ALU.max)
                    pc, pd = psum("bank2"), psum("bank3")
                    nc.tensor.matmul(pc[:], l2a[:, 0:128], h1a[:], start=True, stop=False)
                    nc.tensor.matmul(pc[:], l2b[:, 0:128], h1b[:], start=False, stop=True)
                    nc.tensor.matmul(pd[:], l2a[:, 128:256], h1a[:], start=True, stop=False)
                    nc.tensor.matmul(pd[:], l2b[:, 128:256], h1b[:], start=False, stop=True)
                    h2a = hp.tile([128, 512], F32, tag="h2a")
                    h2b = hp.tile([128, 512], F32, tag="h2b")
                    nc.vector.tensor_scalar(h2a[:], pc[:], bb2[:, 0:1], 0.0,
                                            ALU.add, ALU.max)
                    nc.vector.tensor_scalar(h2b[:], pd[:], bb2[:, 1:2], 0.0,
                                            ALU.add, ALU.max)
                    pe_ = psum("bank4")
                    nc.tensor.matmul(pe_[0:64, :], l3a[:, 0:64], h2a[:], start=True, stop=False)
                    nc.tensor.matmul(pe_[0:64, :], l3b[:, 0:64], h2b[:], start=False, stop=True)
                    oc = out_ap[:, c * 512:(c + 1) * 512]
                    if scale_by is None:
                        nc.vector.tensor_scalar(oc, pe_[0:64, :], bb3[0:64, 0:1],
                                                None, ALU.add)
                    else:
                        w_sb = hp.tile([64, 512], F32, tag="wsb")
                        nc.vector.tensor_scalar(w_sb[:], pe_[0:64, :], bb3[0:64, 0:1],
                                                None, ALU.add)
                        nc.vector.tensor_tensor(
                            oc, w_sb[:], scale_by[:, c * 512:(c + 1) * 512],
                            ALU.mult)

            mlp(kt_t, NK, w1_t, w2a_t, w2b_t, w3a_t, w3b_t, b1_t, b2_t, b3_t,
                ks2[0:64, :], scale_by=kt_t)          # Ks^T into ks2 top
            mlp(qt_t, QSH, w1_t, w2a_t, w2b_t, w3a_t, w3b_t, b1_t, b2_t, b3_t,
                qsct[:], scale_by=qt_t)               # Qs^T
            mlp(qt_t, QSH, u1_t, u2a_t, u2b_t, u3a_t, u3b_t, c1_t, c2_t, c3_t,
                wot[:])                               # Wo^T

            # duplicate Ks^T into bottom half of ks2
            nc.gpsimd.dma_start(ks2[64:128, :], ks2[0:64, :])

            # qs2cols: [128, QSH/2]; col j = [Qs[2j,:] ; Qs[2j+1,:]]
            qs2 = sb.tile([128, QSH // 2], F32, tag="qs2")
            qv = qsct[:].rearrange("p (s h j) -> p s h j", h=2, j=64)
            qd = qs2[:].rearrange("p (s j) -> p s j", j=64)
            nc.gpsimd.dma_start(qd[0:64, :, :], qv[:, :, 0, :])
            nc.gpsimd.dma_start(qd[64:128, :, :], qv[:, :, 1, :])
            nqs2 = sb.tile([128, QSH // 2], F32, tag="nqs2")
            nc.vector.tensor_scalar(nqs2[:], qs2[:], -1.0, None, ALU.mult)

            # row sums Sq [1, QSH], Sk [1, NK] via ones-vector matmuls (fp32)
            ones64 = sb.tile([64, 1], F32, tag="ones64")
            nc.vector.memset(ones64[:], 1.0)
            l_r1 = sb.tile([2, QSH], F32R, tag="l_r1")
            sq_row = sb.tile([1, QSH], F32R, tag="sq_row")
            sq_p = psum("bank5")
            for c in range(QSH // 512):
                nc.tensor.matmul(sq_p[0:1, :], ones64[:],
                                 qsct[:, c * 512:(c + 1) * 512],
                                 start=True, stop=True)
                nc.vector.tensor_copy(sq_row[0:1, c * 512:(c + 1) * 512], sq_p[0:1, :])
            # l_r1[0, s*128 + 2j + h] = Sq[s*128 + 64h + j]
            lv = l_r1[0:1, :].rearrange("p (s j h) -> p s j h", j=64, h=2)
            qv2 = sq_row[0:1, :].rearrange("p (s h j) -> p s h j", h=2, j=64)
            nc.vector.tensor_copy(lv[:, :, :, 0], qv2[:, :, 0, :])
            nc.vector.tensor_copy(lv[:, :, :, 1], qv2[:, :, 1, :])
            rhs_r1 = sb.tile([2, KB * 512], F32R, tag="rhs_r1")
            nc.vector.memset(rhs_r1[0:1, :].bitcast(F32), 1.0)
            nc.gpsimd.dma_start(l_r1[1:2, :], rhs_r1[0:1, 0:QSH])
            sk_p = psum("bank6")
            for c in range(KB):
                nc.tensor.matmul(sk_p[0:1, :], ones64[:],
                                 ks2[0:64, c * 512:(c + 1) * 512],
                                 start=True, stop=True)
                skc = hp.tile([1, 512], F32R, tag="skc", name="skc")
                nc.vector.tensor_scalar(skc[0:1, :], sk_p[0:1, :], -1.0,
                                        None, ALU.mult)
                nc.gpsimd.dma_start(rhs_r1[1:2, c * 512:(c + 1) * 512], skc[0:1, :])

            # Wo natural layout [128, nsub*64] via PE transposes
            wo_nat = sb.tile([128, NSUB * 64], F32, tag="wo_nat")
            for s in range(nsub):
                pt = psum("bank7")
                nc.tensor.matmul(pt[:, 0:64], wot[:, s * 128:(s + 1) * 128],
                                 iden_t[0:64, 0:64], is_transpose=True,
                                 start=True, stop=True)
                nc.vector.tensor_copy(wo_nat[:, s * 64:(s + 1) * 64], pt[:, 0:64])

            out_stage = sb.tile([128, NSUB * 64], F32, tag="out_stage")

            # ---------- main loop over query subtiles ----------
            for s in range(nsub):
                pscore = [psum(f"bank{i}") for i in range(KB)]
                for jp in range(64):
                    col = s * 64 + jp
                    lw = em_t[:, 127 - 2 * jp:255 - 2 * jp]
                    for hf in range(2):
                        ko = hf * (NK // 2)
                        rt = rp.tile([128, NK // 2], F32R, tag="rhs", name="rhs")
                        if (2 * jp + hf) % 8 < 5:
                            nc.vector.tensor_scalar(rt[:], ks2[:, ko:ko + NK // 2],
                                                    qs2[:, col:col + 1], 0.0,
                                                    ALU.subtract, ALU.max)
                        else:
                            nc.scalar.activation(rt[:], ks2[:, ko:ko + NK // 2],
                                                 AF.Relu,
                                                 bias=nqs2[:, col:col + 1], scale=1.0)
                        for kk in range(KB // 2):
                            kb = hf * (KB // 2) + kk
                            nc.tensor.matmul(pscore[kb][:],
                                             lw, rt[:, kk * 512:(kk + 1) * 512],
                                             start=(jp == 0), stop=False)
                for kb in range(KB):
                    nc.tensor.matmul(pscore[kb][:],
                                     l_r1[:, s * 128:(s + 1) * 128],
                                     rhs_r1[:, kb * 512:(kb + 1) * 512],
                                     start=False, stop=True)

                # softmax over keys (free dim): min, square/2, offset, exp
                mcat = hp.tile([128, KB], F32, tag="mcat")
                for kb in range(KB):
                    nc.vector.tensor_reduce(mcat[:, kb:kb + 1], pscore[kb][:],
                                            AX.X, ALU.min)
                m1 = hp.tile([128, 1], F32, tag="m1")
                nc.vector.tensor_reduce(m1[:], mcat[:], AX.X, ALU.min)
                m2h = hp.tile([128, 1], F32, tag="m2h")
                nc.vector.tensor_scalar(m2h[:], m1[:], m1[:, 0:1], 0.5,
                                        ALU.mult, ALU.mult)
                s2h = bp.tile([128, NK], F32, tag="s2h")
                for kb in range(KB):
                    nc.scalar.activation(s2h[:, kb * 512:(kb + 1) * 512],
                                         pscore[kb][:], AF.Square, scale=SQ2)
                uu = bp.tile([128, NK], F32, tag="uu")
                nc.vector.tensor_scalar(uu[:], s2h[:], m2h[:, 0:1], None,
                                        ALU.subtract)

                attn = bp.tile([128, NK], F32, tag="s2h", name="attn")
                for c in range(KCH):
                    put = pp.tile([128, 128], F32, tag=f"bank{c % 8}", name="put")
                    nc.tensor.matmul(put[:], uu[:, c * 128:(c + 1) * 128],
                                     iden_t[:], is_transpose=True,
                                     start=True, stop=True)
                    a_out = attn[:, c * 128:(c + 1) * 128].rearrange(
                        "k (h j) -> k j h", h=2)
                    a_in = put[:].rearrange("k (j h) -> k j h", h=2)
                    nc.scalar.activation(a_out, a_in, AF.Exp, scale=-1.0)

                pctx = pp.tile([65, 128], F32, tag="bank0", name="pctx")
                for c in range(KCH):
                    nc.tensor.matmul(pctx[:], v1_t[:, c * 65:(c + 1) * 65],
                                     attn[:, c * 128:(c + 1) * 128],
                                     start=(c == 0), stop=(c == KCH - 1))
                ctxs = hp.tile([65, 128], F32, tag="ctxs")
                nc.vector.tensor_copy(ctxs[:], pctx[:])
                pctx2 = pp.tile([128, 65], F32, tag="bank1", name="pctx2")
                nc.tensor.matmul(pctx2[:], ctxs[:], iden_t[0:65, 0:65],
                                 is_transpose=True, start=True, stop=True)
                rcp = hp.tile([128, 1], F32, tag="rcp")
                nc.vector.reciprocal(rcp[:], pctx2[:, 64:65])
                tmpo = hp.tile([128, 64], F32, tag="tmpo")
                nc.vector.tensor_scalar(tmpo[:], pctx2[:, 0:64], rcp[:, 0:1],
                                        None, ALU.mult)
                nc.vector.tensor_tensor(out_stage[:, s * 64:(s + 1) * 64],
                                        tmpo[:], wo_nat[:, s * 64:(s + 1) * 64],
                                        ALU.mult)

            ov = O.rearrange("(s p) f -> p s f", p=128)
            sv = out_stage[:].rearrange("p (s f) -> p s f", f=64)
            nc.sync.dma_start(ov[:, 0:nsub, :], sv[:, 0:nsub, :])

    nc.compile()
    return nc


def _host_prep(inputs, core, nsub):
    """Build the per-core input map (host-side layout prep only)."""
    b = core // 2
    qh = core % 2
    K = inputs["KEY"][b]                      # [NK, 64]
    Q = inputs["QUERY"][b][qh * QSH:(qh + 1) * QSH]
    V = inputs["VALUE"][b]
    v1 = np.concatenate([V, np.ones((NK, 1), np.float32)], axis=1)  # [NK, 65]
    em = np.zeros((128, 256), np.float32)
    em[0:64, 127] = 2.0
    em[64:128, 128] = 2.0
    m = {
        "kt": np.ascontiguousarray(K.T),
        "qt": np.ascontiguousarray(Q.T),
        "v1": np.ascontiguousarray(
            v1.reshape(KCH, 128, 65).transpose(1, 0, 2).reshape(128, KCH * 65)),
        "w1": np.ascontiguousarray(inputs["W1_w"].T),
        "w2a": np.ascontiguousarray(inputs["W2_w"].T[0:128]),
        "w2b": np.ascontiguousarray(inputs["W2_w"].T[128:256]),
        "w3a": np.ascontiguousarray(inputs["W3_w"].T[0:128]),
        "w3b": np.ascontiguousarray(inputs["W3_w"].T[128:256]),
        "b1c": np.ascontiguousarray(inputs["W1_b"].reshape(2, 128).T),
        "b2c": np.ascontiguousarray(inputs["W2_b"].reshape(2, 128).T),
        "b3c": np.ascontiguousarray(
            np.pad(inputs["W3_b"], (0, 64)).reshape(1, 128).T),
        "u1": np.ascontiguousarray(inputs["Wo1_w"].T),
        "u2a": np.ascontiguousarray(inputs["Wo2_w"].T[0:128]),
        "u2b": np.ascontiguousarray(inputs["Wo2_w"].T[128:256]),
        "u3a": np.ascontiguousarray(inputs["Wo3_w"].T[0:128]),
        "u3b": np.ascontiguousarray(inputs["Wo3_w"].T[128:256]),
        "c1c": np.ascontiguousarray(inputs["Wo1_b"].reshape(2, 128).T),
        "c2c": np.ascontiguousarray(inputs["Wo2_b"].reshape(2, 128).T),
        "c3c": np.ascontiguousarray(
            np.pad(inputs["Wo3_b"], (0, 64)).reshape(1, 128).T),
        "em": em,
        "iden": np.eye(128, dtype=np.float32),
    }
    return {k: np.ascontiguousarray(v.astype(np.float32)) for k, v in m.items()}


def run(inputs, nsub=NSUB, trace=False):
    if nsub not in _cache:
        _cache[nsub] = _build(nsub)
    nc = _cache[nsub]
    in_maps = [_host_prep(inputs, c, nsub) for c in range(NCORES)]
    res = run_bass_kernel_spmd(nc, in_maps, list(range(NCORES)), trace=trace)
    out = np.zeros((B, NQ, DV), np.float32)
    for c in range(NCORES):
        b, qh = c // 2, c % 2
        out[b, qh * QSH:qh * QSH + nsub * 128] = \
            res.results[c]["o"][0:nsub * 128]
    return out, res


def kernel(**inputs):
    out, _ = run(inputs)
    return out



# revision 6
# speedup vs baseline: 1.3562x; 1.3562x over previous
"""Trainium2 Bass kernel for the L1-distance attention + MLP-scaling model.

Math (per batch b):
  Wk = MLP(K), Wq = MLP(Q), Wo = MLPo(Q)
  Ks = K*Wk, Qs = Q*Wq
  score[k,q] = sum_d |Ks[k,d] - Qs[q,d]|
             = (Sq[q] - Sk[k]) + 2*sum_d relu(Ks[k,d] - Qs[q,d])
  attn = softmax_k(-(score^2)/2)
  out = (attn^T @ V) * Wo

Sharding: 8 cores = 4 batches x 2 query-halves. Each core handles all 4096
keys and 2048 queries of its batch.

Implementation notes (v2, fp16 score path):
  * The score pipeline (Ks/Qs/relu-diff/em-reduction) runs in fp16: the DVE
    gets its 4x packed-16-bit mode for the relu ops and the PE streams the
    reduction matmuls at 1 col/cycle.
  * Per query-subtile (128 queries), scores for all 4096 keys live in the 8
    PSUM banks. Keys are split 3072 (banks 0-5) + 1024 (banks 6-7): the
    banks 6-7 matmuls are emitted after all banks 0-5 work so that the
    previous subtile's context accumulation can borrow bank 7 while the
    current subtile's main phase runs -- this hides the softmax/context tail
    inside the next subtile's score phase.
  * The attn transpose (for the context matmul) uses the DMA XBAR transpose
    (16-bit only) instead of PE transposes, keeping PE free for scores.
  * exp is fused with the max-shift via the activation bias input.
"""
import sys
sys.path.insert(0, '/opt/trn_rl_repo')
import numpy as np
from contextlib import ExitStack

import concourse.bass as bass
import concourse.bacc as bacc
import concourse.tile as tile
from concourse import mybir
from concourse.bass_utils import run_bass_kernel_spmd

dt = mybir.dt
F32 = dt.float32
F32R = dt.float32r
F16 = dt.float16
ALU = mybir.AluOpType
AF = mybir.ActivationFunctionType
AX = mybir.AxisListType

B, NK, NQ, DK, DV, H = 4, 4096, 4096, 64, 64, 256
NCORES = 8
QSH = NQ // 2            # queries per core
NSUB = QSH // 128        # 16 q-subtiles of 128
KCH = NK // 128          # 32 key chunks
KB = NK // 512           # 8 psum banks of 512 keys
KB_A = 6                 # banks computed in the main phase
NKA = KB_A * 512         # 3072 keys in main phase
SQ2 = float(np.float32(1.0 / np.sqrt(2.0)))

_cache = {}


def _build(nsub, reps=1):
    nc = bacc.Bacc("TRN2", target_bir_lowering=False, debug=False,
                   num_devices=NCORES)

    def din(name, shape, d=F16):
        return nc.dram_tensor(name, shape, d, kind="ExternalInput").ap()

    kt = din("kt", [64, NK])            # K^T (fp16)
    qt = din("qt", [64, QSH])           # Q^T (fp16)
    v1 = din("v1", [128, KCH * 65])     # [V | ones] packed per key-chunk
    w1 = din("w1", [64, H])             # W1_w^T
    w2a = din("w2a", [128, H]); w2b = din("w2b", [128, H])
    w3a = din("w3a", [128, DK]); w3b = din("w3b", [128, DK])
    b1c = din("b1c", [128, 2], F32); b2c = din("b2c", [128, 2], F32)
    b3c = din("b3c", [128, 1], F32)
    u1 = din("u1", [64, H])
    u2a = din("u2a", [128, H]); u2b = din("u2b", [128, H])
    u3a = din("u3a", [128, DK]); u3b = din("u3b", [128, DK])
    c1c = din("c1c", [128, 2], F32); c2c = din("c2c", [128, 2], F32)
    c3c = din("c3c", [128, 1], F32)
    em = din("em", [128, 64 * 128])     # per-jp E tiles (+2.0 cols), fp16
    iden = din("iden", [128, 128])      # fp16 identity for PE transposes
    O = nc.dram_tensor("o", [QSH, DV], F32, kind="ExternalOutput").ap()

    with tile.TileContext(nc) as tc:
        with ExitStack() as ctx:
            if reps > 1:
                ctx.enter_context(tc.For_i(0, reps, 1))
            sb = ctx.enter_context(tc.tile_pool(name="sb", bufs=1))
            hp = ctx.enter_context(tc.tile_pool(name="hp", bufs=2))
            rp = ctx.enter_context(tc.tile_pool(name="rp", bufs=2))
            rbp = ctx.enter_context(tc.tile_pool(name="rbp", bufs=2))
            pp = ctx.enter_context(tc.tile_pool(name="pp", bufs=1, space="PSUM"))

            def psum(tag, shape=(128, 512), d=F32, name=None):
                return pp.tile(list(shape), d, tag=tag, name=name or tag)

            # ---------- load inputs ----------
            def load(ap_dram, shape, d=F16, tag=None):
                t = sb.tile(shape, d, tag=tag, name=tag)
                nc.gpsimd.dma_start(t[:], ap_dram)
                return t

            kt_t = load(kt, [64, NK], tag="kt")
            qt_t = load(qt, [64, QSH], tag="qt")
            v1_t = load(v1, [128, KCH * 65], tag="v1")
            iden_t = load(iden, [128, 128], tag="iden")
            em_t = load(em, [128, 64 * 128], tag="em_t")

            w1_t = load(w1, [64, H], tag="w1")
            w2a_t = load(w2a, [128, H], tag="w2a")
            w2b_t = load(w2b, [128, H], tag="w2b")
            w3a_t = load(w3a, [128, DK], tag="w3a")
            w3b_t = load(w3b, [128, DK], tag="w3b")
            u1_t = load(u1, [64, H], tag="u1")
            u2a_t = load(u2a, [128, H], tag="u2a")
            u2b_t = load(u2b, [128, H], tag="u2b")
            u3a_t = load(u3a, [128, DK], tag="u3a")
            u3b_t = load(u3b, [128, DK], tag="u3b")
            b1_t = load(b1c, [128, 2], F32, tag="b1")
            b2_t = load(b2c, [128, 2], F32, tag="b2")
            b3_t = load(b3c, [128, 1], F32, tag="b3")
            c1_t = load(c1c, [128, 2], F32, tag="c1")
            c2_t = load(c2c, [128, 2], F32, tag="c2")
            c3_t = load(c3c, [128, 1], F32, tag="c3")

            # ---------- MLPs (transposed layout: features on partitions) ----------
            ks2 = sb.tile([128, NK], F16, tag="ks2")
            qsct = sb.tile([64, QSH], F16, tag="qsct")
            wot = sb.tile([64, QSH], F16, tag="wot")

            def mlp(x_t, T, l1, l2a, l2b, l3a, l3b, bb1, bb2, bb3, out_ap,
                    scale_by=None):
                # x_t: [64, T] input^T (fp16); writes MLP output^T (64 rows)
                # to out_ap; if scale_by is given, writes (mlp_out * scale_by)
                for c in range(T // 512):
                    xc = x_t[:, c * 512:(c + 1) * 512]
                    pa = psum("bank0"); pb = psum("bank1")
                    nc.tensor.matmul(pa[:], l1[:, 0:128], xc, start=True, stop=True)
                    nc.tensor.matmul(pb[:], l1[:, 128:256], xc, start=True, stop=True)
                    h1a = hp.tile([128, 512], F16, tag="h1a")
                    h1b = hp.tile([128, 512], F16, tag="h1b")
                    nc.vector.tensor_scalar(h1a[:], pa[:], bb1[:, 0:1], 0.0,
                                            ALU.add, ALU.max)
                    nc.scalar.activation(h1b[:], pb[:], AF.Relu,
                                         bias=bb1[:, 1:2], scale=1.0)
                    pc = psum("bank2"); pd = psum("bank3")
                    nc.tensor.matmul(pc[:], l2a[:, 0:128], h1a[:], start=True, stop=False)
                    nc.tensor.matmul(pc[:], l2b[:, 0:128], h1b[:], start=False, stop=True)
                    nc.tensor.matmul(pd[:], l2a[:, 128:256], h1a[:], start=True, stop=False)
                    nc.tensor.matmul(pd[:], l2b[:, 128:256], h1b[:], start=False, stop=True)
                    h2a = hp.tile([128, 512], F16, tag="h2a")
                    h2b = hp.tile([128, 512], F16, tag="h2b")
                    nc.vector.tensor_scalar(h2a[:], pc[:], bb2[:, 0:1], 0.0,
                                            ALU.add, ALU.max)
                    nc.scalar.activation(h2b[:], pd[:], AF.Relu,
                                         bias=bb2[:, 1:2], scale=1.0)
                    pe_ = psum("bank4")
                    nc.tensor.matmul(pe_[0:64, :], l3a[:, 0:64], h2a[:], start=True, stop=False)
                    nc.tensor.matmul(pe_[0:64, :], l3b[:, 0:64], h2b[:], start=False, stop=True)
                    oc = out_ap[:, c * 512:(c + 1) * 512]
                    if scale_by is None:
                        nc.vector.tensor_scalar(oc, pe_[0:64, :], bb3[0:64, 0:1],
                                                None, ALU.add)
                    else:
                        w_sb = hp.tile([64, 512], F32, tag="wsb")
                        nc.vector.tensor_scalar(w_sb[:], pe_[0:64, :], bb3[0:64, 0:1],
                                                None, ALU.add)
                        nc.vector.tensor_tensor(
                            oc, w_sb[:], scale_by[:, c * 512:(c + 1) * 512],
                            ALU.mult)

            mlp(kt_t, NK, w1_t, w2a_t, w2b_t, w3a_t, w3b_t, b1_t, b2_t, b3_t,
                ks2[0:64, :], scale_by=kt_t)          # Ks^T into ks2 top
            mlp(qt_t, QSH, w1_t, w2a_t, w2b_t, w3a_t, w3b_t, b1_t, b2_t, b3_t,
                qsct[:], scale_by=qt_t)               # Qs^T
            mlp(qt_t, QSH, u1_t, u2a_t, u2b_t, u3a_t, u3b_t, c1_t, c2_t, c3_t,
                wot[:])                               # Wo^T

            # duplicate Ks^T into bottom half of ks2
            nc.gpsimd.dma_start(ks2[64:128, :], ks2[0:64, :])

            # qs2: [128, QSH/2]; col j = [Qs[2j,:] ; Qs[2j+1,:]]
            qs2h = sb.tile([128, QSH // 2], F16, tag="qs2h")
            qv = qsct[:].rearrange("p (s h j) -> p s h j", h=2, j=64)
            qd = qs2h[:].rearrange("p (s j) -> p s j", j=64)
            nc.gpsimd.dma_start(qd[0:64, :, :], qv[:, :, 0, :])
            nc.gpsimd.dma_start(qd[64:128, :, :], qv[:, :, 1, :])
            # f32 copy (tensor_scalar scalar operands must be f32)
            qs2 = sb.tile([128, QSH // 2], F32, tag="qs2")
            nc.vector.tensor_copy(qs2[:], qs2h[:])

            # row sums Sq [1, QSH], Sk [1, NK] via ones-vector matmuls
            ones64 = sb.tile([64, 1], F16, tag="ones64")
            nc.vector.memset(ones64[:], 1.0)
            l_r1 = sb.tile([2, QSH], F16, tag="l_r1")
            sq_p = psum("bank5")
            for c in range(QSH // 512):
                nc.tensor.matmul(sq_p[0:1, :], ones64[:],
                                 qsct[:, c * 512:(c + 1) * 512],
                                 start=True, stop=True)
                nc.vector.tensor_copy(l_r1[0:1, c * 512:(c + 1) * 512], sq_p[0:1, :])
            rhs_r1 = sb.tile([2, KB * 512], F16, tag="rhs_r1")
            nc.vector.memset(rhs_r1[0:1, :], 1.0)
            nc.gpsimd.dma_start(l_r1[1:2, :], rhs_r1[0:1, 0:QSH])
            sk_p = psum("bank6")
            for c in range(KB):
                nc.tensor.matmul(sk_p[0:1, :], ones64[:],
                                 ks2[0:64, c * 512:(c + 1) * 512],
                                 start=True, stop=True)
                skc = hp.tile([1, 512], F16, tag="skc", name="skc")
                nc.vector.tensor_scalar(skc[0:1, :], sk_p[0:1, :], -1.0,
                                        None, ALU.mult)
                nc.gpsimd.dma_start(rhs_r1[1:2, c * 512:(c + 1) * 512], skc[0:1, :])

            # Wo natural layout [128, nsub*64] via PE transposes (fp16)
            wo_nat = sb.tile([128, NSUB * 64], F16, tag="wo_nat")
            for s in range(nsub):
                pt = psum("bank7", (128, 128), F16, name="pt")
                nc.tensor.matmul(pt[:, 0:64], wot[:, s * 128:(s + 1) * 128],
                                 iden_t[0:64, 0:64], is_transpose=True,
                                 start=True, stop=True)
                nc.vector.tensor_copy(wo_nat[:, s * 64:(s + 1) * 64], pt[:, 0:64])

            out_stage = sb.tile([128, NSUB * 64], F32, tag="out_stage")

            # persistent per-subtile tiles for the tail pipeline
            s2h = sb.tile([128, NK], F32, tag="s2h")
            attn = sb.tile([128, NK], F16, tag="attn")
            attnT = sb.tile([128, NK], F16, tag="attnT")
            mcat = sb.tile([128, KB], F32, tag="mcat")
            m1 = sb.tile([128, 1], F32, tag="m1")

            # ---------- main loop over query subtiles ----------
            def emit_score_main(s, jp_lo, jp_hi):
                # banks 0..KB_A-1 (keys 0..NKA)
                for jp in range(jp_lo, jp_hi):
                    col = s * 64 + jp
                    lw = em_t[:, jp * 128:(jp + 1) * 128]
                    rt = rp.tile([128, NKA], F16, tag="rta", name="rta")
                    nc.vector.tensor_scalar(rt[:], ks2[:, 0:NKA],
                                            qs2[:, col:col + 1], 0.0,
                                            ALU.subtract, ALU.max)
                    for kb in range(KB_A):
                        nc.tensor.matmul(pscore[kb][:],
                                         lw, rt[:, kb * 512:(kb + 1) * 512],
                                         start=(jp == 0), stop=False)

            def emit_score_tail_banks(s):
                # banks KB_A..KB-1 (keys NKA..NK), deferred
                for jp in range(64):
                    col = s * 64 + jp
                    lw = em_t[:, jp * 128:(jp + 1) * 128]
                    rt = rbp.tile([128, NK - NKA], F16, tag="rtb", name="rtb")
                    nc.vector.tensor_scalar(rt[:], ks2[:, NKA:NK],
                                            qs2[:, col:col + 1], 0.0,
                                            ALU.subtract, ALU.max)
                    for kb in range(KB_A, KB):
                        nc.tensor.matmul(pscore[kb][:],
                                         lw, rt[:, (kb - KB_A) * 512:
                                                 (kb - KB_A + 1) * 512],
                                         start=(jp == 0), stop=False)

            def emit_r1_square(s, kb_lo, kb_hi):
                for kb in range(kb_lo, kb_hi):
                    nc.tensor.matmul(pscore[kb][:],
                                     l_r1[:, s * 128:(s + 1) * 128],
                                     rhs_r1[:, kb * 512:(kb + 1) * 512],
                                     start=False, stop=True)
                for kb in range(kb_lo, kb_hi):
                    # s2h = (score/sqrt(2))^2 = score^2/2  (frees the bank)
                    nc.scalar.activation(s2h[:, kb * 512:(kb + 1) * 512],
                                         pscore[kb][:], AF.Square, scale=SQ2)
                    nc.vector.tensor_reduce(mcat[:, kb:kb + 1],
                                            s2h[:, kb * 512:(kb + 1) * 512],
                                            AX.X, ALU.min)

            def emit_softmax(s):
                nc.vector.tensor_reduce(m1[:], mcat[:], AX.X, ALU.min)
                for c in range(KB):
                    # attn = exp(m1 - s2h)  (max-shifted softmax numerator)
                    nc.scalar.activation(attn[:, c * 512:(c + 1) * 512],
                                         s2h[:, c * 512:(c + 1) * 512],
                                         AF.Exp, bias=m1[:, 0:1], scale=-1.0)
                for c in range(KCH):
                    nc.sync.dma_start_transpose(
                        attnT[:, c * 128:(c + 1) * 128],
                        attn[:, c * 128:(c + 1) * 128])

            ctx_state = {}

            def emit_ctx_mm(s):
                # context for subtile s (borrows bank7 during s+1's main phase)
                pctx = psum("bank7", (65, 128), name="pctx")
                for c in range(KCH):
                    nc.tensor.matmul(pctx[:], v1_t[:, c * 65:(c + 1) * 65],
                                     attnT[:, c * 128:(c + 1) * 128],
                                     start=(c == 0), stop=(c == KCH - 1))
                ctxs = hp.tile([65, 128], F16, tag="ctxs")
                nc.scalar.activation(ctxs[:], pctx[:], AF.Copy, scale=1.0)
                pctx2 = psum("bank7", (128, 65), F16, name="pctx2")
                nc.tensor.matmul(pctx2[:], ctxs[:], iden_t[0:65, 0:65],
                                 is_transpose=True, start=True, stop=True)
                ctx_state["pctx2"] = pctx2

            def emit_ctx_fin(s):
                pctx2 = ctx_state.pop("pctx2")
                rcp = hp.tile([128, 1], F32, tag="rcp")
                nc.vector.reciprocal(rcp[:], pctx2[:, 64:65])
                tmpo = hp.tile([128, 64], F32, tag="tmpo")
                nc.vector.tensor_scalar(tmpo[:], pctx2[:, 0:64], rcp[:, 0:1],
                                        None, ALU.mult)
                nc.vector.tensor_tensor(out_stage[:, s * 64:(s + 1) * 64],
                                        tmpo[:], wo_nat[:, s * 64:(s + 1) * 64],
                                        ALU.mult)

            pscore = [None] * KB
            for s in range(nsub):
                for i in range(KB_A):
                    pscore[i] = psum(f"bank{i}", name=f"pscore{i}")
                emit_score_main(s, 0, 8)
                if s > 0:
                    emit_ctx_mm(s - 1)
                emit_score_main(s, 8, 16)
                if s > 0:
                    emit_ctx_fin(s - 1)
                emit_score_main(s, 16, 64)
                emit_r1_square(s, 0, KB_A)
                for i in range(KB_A, KB):
                    pscore[i] = psum(f"bank{i}", name=f"pscore{i}")
                emit_score_tail_banks(s)
                emit_r1_square(s, KB_A, KB)
                emit_softmax(s)
            emit_ctx_mm(nsub - 1)
            emit_ctx_fin(nsub - 1)

            ov = O.rearrange("(s p) f -> p s f", p=128)
            sv = out_stage[:].rearrange("p (s f) -> p s f", f=64)
            nc.sync.dma_start(ov[:, 0:nsub, :], sv[:, 0:nsub, :])

    nc.compile()
    return nc


def _host_prep(inputs, core, nsub):
    """Build the per-core input map (host-side layout prep only)."""
    b = core // 2
    qh = core % 2
    K = inputs["KEY"][b]                      # [NK, 64]
    Q = inputs["QUERY"][b][qh * QSH:(qh + 1) * QSH]
    V = inputs["VALUE"][b]
    v1 = np.concatenate([V, np.ones((NK, 1), np.float32)], axis=1)  # [NK, 65]
    em = np.zeros((128, 64 * 128), np.float16)
    for jp in range(64):
        em[0:64, jp * 128 + jp] = 2.0
        em[64:128, jp * 128 + 64 + jp] = 2.0
    f16 = np.float16
    m = {
        "kt": K.T.astype(f16),
        "qt": Q.T.astype(f16),
        "v1": v1.reshape(KCH, 128, 65).transpose(1, 0, 2).reshape(
            128, KCH * 65).astype(f16),
        "w1": inputs["W1_w"].T.astype(f16),
        "w2a": inputs["W2_w"].T[0:128].astype(f16),
        "w2b": inputs["W2_w"].T[128:256].astype(f16),
        "w3a": inputs["W3_w"].T[0:128].astype(f16),
        "w3b": inputs["W3_w"].T[128:256].astype(f16),
        "b1c": inputs["W1_b"].reshape(2, 128).T.astype(np.float32),
        "b2c": inputs["W2_b"].reshape(2, 128).T.astype(np.float32),
        "b3c": np.pad(inputs["W3_b"], (0, 64)).reshape(1, 128).T.astype(
            np.float32),
        "u1": inputs["Wo1_w"].T.astype(f16),
        "u2a": inputs["Wo2_w"].T[0:128].astype(f16),
        "u2b": inputs["Wo2_w"].T[128:256].astype(f16),
        "u3a": inputs["Wo3_w"].T[0:128].astype(f16),
        "u3b": inputs["Wo3_w"].T[128:256].astype(f16),
        "c1c": inputs["Wo1_b"].reshape(2, 128).T.astype(np.float32),
        "c2c": inputs["Wo2_b"].reshape(2, 128).T.astype(np.float32),
        "c3c": np.pad(inputs["Wo3_b"], (0, 64)).reshape(1, 128).T.astype(
            np.float32),
        "em": em,
        "iden": np.eye(128, dtype=f16),
    }
    return {k: np.ascontiguousarray(v) for k, v in m.items()}


def run(inputs, nsub=NSUB, trace=False):
    if nsub not in _cache:
        _cache[nsub] = _build(nsub)
    nc = _cache[nsub]
    in_maps = [_host_prep(inputs, c, nsub) for c in range(NCORES)]
    res = run_bass_kernel_spmd(nc, in_maps, list(range(NCORES)), trace=trace)
    out = np.zeros((B, NQ, DV), np.float32)
    for c in range(NCORES):
        b, qh = c // 2, c % 2
        out[b, qh * QSH:qh * QSH + nsub * 128] = \
            res.results[c]["o"][0:nsub * 128]
    return out, res


def kernel(**inputs):
    out, _ = run(inputs)
    return out
